# revision 4
# baseline (speedup 1.0000x reference)
"""AdaptiveSAGE GNN message-passing kernel for 8 Trainium2 NeuronCores.

Distribution strategy (dst-sharded message passing, PE-based segment sum):
  - Subgraph nodes padded to N_PAD = 81920 = 8 * 10240; core c owns rows
    [c*10240, (c+1)*10240).
  - Edges are assigned to the core owning their destination, bucketed by
    (src bank, dst window of 128 rows), and padded per bucket to a multiple
    of 128 so all cores run one identical instruction stream.
  - Each core gathers H[src] messages with dma_gather (int16 indices =>
    the 81920-row replicated H table is split into 3 banks <= 32768 rows).
  - Segment-sum by destination runs on the TensorEngine: per 128-message
    tile a one-hot(dst) matrix (built by a batched DVE is_equal against an
    iota row) is the stationary matmul operand; PSUM accumulates the
    window's aggregate exactly (no DMA scatter -> no RMW races).
  - Per-core SAGE update is computed feature-major on the TensorEngine,
    then the new H rows are AllGathered into the replicated H table.
  - The tiny MLP heads (scores / halting probs on rows u=0, v=1) are
    evaluated on the host from the 5 x 2 x 128 head rows the kernel emits.
"""

import math

import numpy as np

import concourse.bass as bass
import concourse.bacc as bacc
import concourse.tile as tile
import concourse.mybir as mybir
from concourse.bass_utils import run_bass_kernel_spmd

F = 128          # feature dim
N_CORES = 8
N_SUB = 80000
PER = 10240      # rows per core
N_PAD = N_CORES * PER
W = PER // F     # dst windows of 128 rows per core
LMAX = 5
CHUNK = 1024     # messages per gather chunk (SWDGE ring caps num_idxs ~<2K)
PAD_DL = 999.0   # out-of-window dst marker for padding slots

# gather banks (each <= 32767 rows so indices fit int16)
BANKS = [(0, 27648), (27648, 27648), (55296, N_PAD - 55296)]


def _wrap16(idx: np.ndarray) -> np.ndarray:
    """SWDGE index layout: logical i -> [i%16, i//16], replicated across the
    8 groups of 16 partitions."""
    n = idx.shape[0]
    assert n % 16 == 0
    w = idx.reshape(n // 16, 16).T.astype(np.int16)
    return np.tile(w, (8, 1))


def _prep_edges(src: np.ndarray, dst: np.ndarray):
    """Bucket edges by (dst core, src bank, dst window); pad each bucket to a
    common multiple-of-128 so the SPMD graph is uniform across cores.

    Returns per-core gather index arrays (3 banks, wrapped int16), per-core
    one-hot dst columns dl [128, T_total] f32, per-bank padded slot counts L,
    and the static run list [(bank, window, n_tiles)].
    """
    nw = W
    core_of = dst // PER
    bank_of = np.digitize(src, [BANKS[1][0], BANKS[2][0]])
    dst_local = dst - core_of * PER
    w_of = dst_local // F
    run_of = bank_of * nw + w_of

    counts = np.zeros((N_CORES, 3 * nw), dtype=np.int64)
    per_core = []
    for c in range(N_CORES):
        m = core_of == c
        gl = (src[m] - np.array([b[0] for b in BANKS])[bank_of[m]]).astype(np.int16)
        dl = (dst_local[m] % F).astype(np.float32)
        rid = run_of[m]
        order = np.argsort(rid, kind="stable")
        gl, dl, rid = gl[order], dl[order], rid[order]
        bounds = np.searchsorted(rid, np.arange(3 * nw + 1))
        counts[c] = bounds[1:] - bounds[:-1]
        per_core.append((gl, dl, bounds))

    nt = np.ceil(counts.max(axis=0) / F).astype(np.int64)  # tiles per run
    runs = [(r // nw, r % nw, int(nt[r])) for r in range(3 * nw) if nt[r] > 0]
    # per-bank padded slot counts and per-run slot offsets
    L = [0, 0, 0]
    slot0 = {}
    tile0 = {}
    tg = 0
    for (b, w, n) in runs:
        slot0[(b, w)] = L[b]
        tile0[(b, w)] = tg
        L[b] += n * F
        tg += n
    T_total = tg

    gidx, dls = [], []
    for c in range(N_CORES):
        gl, dl, bounds = per_core[c]
        gb = [np.zeros(L[b], np.int16) for b in range(3)]
        dla = np.full(T_total * F, PAD_DL, np.float32)
        for (b, w, n) in runs:
            r = b * nw + w
            seg = slice(bounds[r], bounds[r + 1])
            cnt = bounds[r + 1] - bounds[r]
            s0 = slot0[(b, w)]
            gb[b][s0:s0 + cnt] = gl[seg]
            t0 = tile0[(b, w)]
            dla[t0 * F:t0 * F + cnt] = dl[seg]
        gidx.append([_wrap16(x) for x in gb])
        dls.append(np.ascontiguousarray(dla.reshape(T_total, F).T))
    return gidx, dls, L, runs, T_total


def _build_graph(L, runs, T_total):
    """Build the SPMD Bass graph (identical for all 8 cores)."""
    f32 = mybir.dt.float32
    i16 = mybir.dt.int16
    nt_max = max(n for (_, _, n) in runs)
    # first contributing bank per window (copy vs accumulate)
    first_bank = {}
    for (b, w, n) in runs:
        first_bank.setdefault(w, b)

    nc = bacc.Bacc("TRN2", target_bir_lowering=False, debug=False,
                   num_devices=N_CORES)

    # ---- kernel I/O -------------------------------------------------------
    xT = nc.dram_tensor("xT", [F, PER], f32, kind="ExternalInput")
    invdeg = nc.dram_tensor("invdeg", [F, W], f32, kind="ExternalInput")
    winT = nc.dram_tensor("winT", [F, F], f32, kind="ExternalInput")
    wlT = nc.dram_tensor("wlT", [F, F], f32, kind="ExternalInput")
    wrT = nc.dram_tensor("wrT", [F, F], f32, kind="ExternalInput")
    bin_ = nc.dram_tensor("bin", [F, 1], f32, kind="ExternalInput")
    bl = nc.dram_tensor("bl", [F, 1], f32, kind="ExternalInput")
    ident = nc.dram_tensor("ident", [F, F], f32, kind="ExternalInput")
    iotar = nc.dram_tensor("iotar", [F, F], f32, kind="ExternalInput")
    dl_d = nc.dram_tensor("dl", [F, T_total], f32, kind="ExternalInput")
    gidx_d = [nc.dram_tensor(f"gidx{b}", [128, L[b] // 16], i16,
                             kind="ExternalInput") for b in range(3)]
    out = nc.dram_tensor("out", [LMAX, 2, F], f32, kind="ExternalOutput")

    # ---- internal DRAM ----------------------------------------------------
    h_table = nc.dram_tensor("h_table", [N_PAD, F], f32, addr_space="Shared")
    ag_in = nc.dram_tensor("ag_in", [PER, F], f32)

    rg = [list(range(N_CORES))]

    with tile.TileContext(nc) as tc:
        with (
            tc.tile_pool(name="sb", bufs=1) as sb,
            tc.tile_pool(name="msgp", bufs=4) as msgp,
            tc.tile_pool(name="ohp", bufs=3) as ohp,
            tc.tile_pool(name="aggtp", bufs=2) as aggtp,
            tc.tile_pool(name="psw", bufs=4, space="PSUM") as pswp,
            tc.tile_pool(name="psh", bufs=2, space="PSUM") as pshp,
            tc.tile_pool(name="psr", bufs=2, space="PSUM") as psrp,
        ):
            # persistent SBUF
            HT = sb.tile([F, PER], f32, tag="HT")       # H local, feature-major
            AGG = sb.tile([128, W, F], f32, tag="AGG")  # agg rows / hrow staging
            w_in = sb.tile([F, F], f32, tag="w_in")
            w_l = sb.tile([F, F], f32, tag="w_l")
            w_r = sb.tile([F, F], f32, tag="w_r")
            b_in = sb.tile([F, 1], f32, tag="b_in")
            b_l = sb.tile([F, 1], f32, tag="b_l")
            idn = sb.tile([F, F], f32, tag="idn")
            iot = sb.tile([F, F], f32, tag="iot")
            ivd = sb.tile([F, W], f32, tag="ivd")
            dlsb = sb.tile([F, T_total], f32, tag="dlsb")
            gsb = [sb.tile([128, L[b] // 16], i16, tag=f"g{b}", name=f"g{b}")
                   for b in range(3)]

            # ---- stage 0: loads -------------------------------------------
            nc.sync.dma_start(w_in[:], winT[:, :])
            nc.sync.dma_start(w_l[:], wlT[:, :])
            nc.sync.dma_start(w_r[:], wrT[:, :])
            nc.sync.dma_start(b_in[:], bin_[:, :])
            nc.sync.dma_start(b_l[:], bl[:, :])
            nc.sync.dma_start(idn[:], ident[:, :])
            nc.sync.dma_start(iot[:], iotar[:, :])
            nc.sync.dma_start(ivd[:], invdeg[:, :])
            nc.sync.dma_start(dlsb[:], dl_d[:, :])
            for b in range(3):
                nc.sync.dma_start(gsb[b][:], gidx_d[b][:, :])

            # xT -> AGG viewed feature-major [F, PER]
            AGGf = AGG[:].rearrange("p w f -> p (w f)")
            nc.sync.dma_start(AGGf, xT[:, :])

            # H0 = W_in @ xT + b_in (feature-major), then row-major for AG
            for w in range(W):
                ws = slice(w * F, (w + 1) * F)
                ph = pshp.tile([F, F], f32, tag="psh")
                nc.tensor.matmul(ph[:], lhsT=w_in[:], rhs=AGGf[:, ws],
                                 start=True, stop=True)
                nc.vector.tensor_scalar_add(HT[:, ws], ph[:], b_in[:, 0:1])
            for w in range(W):
                ws = slice(w * F, (w + 1) * F)
                pr = psrp.tile([F, F], f32, tag="psr")
                nc.tensor.transpose(pr[:], HT[:, ws], idn[:])
                nc.vector.tensor_copy(AGG[:, w, :], pr[:])
            nc.sync.dma_start(ag_in[:, :].rearrange("(w p) f -> p w f", p=128),
                              AGG[:])
            nc.gpsimd.collective_compute(
                "AllGather", mybir.AluOpType.bypass, replica_groups=rg,
                ins=[ag_in.ap().opt()], outs=[h_table.ap().opt()])

            # ---- steps ----------------------------------------------------
            for k in range(LMAX):
                last = k == LMAX - 1
                # chunked gathers, bank-major slot order
                msg_tiles = {}
                for b in range(3):
                    base, rows = BANKS[b]
                    bank_ap = h_table[base:base + rows, :]
                    for j0 in range(0, L[b], CHUNK):
                        n = min(CHUNK, L[b] - j0)
                        msg = msgp.tile([128, CHUNK // 128, F], f32, tag="msg",
                                        name=f"msg_{k}_{b}_{j0}")
                        cols = slice(j0 // 16, (j0 + n) // 16)
                        nc.gpsimd.dma_gather(
                            out_ap=msg[:, : n // 128, :], in_ap=bank_ap,
                            idxs_ap=gsb[b][:, cols],
                            num_idxs=n, num_idxs_reg=n, elem_size=F)
                        msg_tiles[(b, j0 // CHUNK)] = msg

                # segment-sum per (bank, window) run on the TensorEngine
                tg = 0
                slot = [0, 0, 0]
                for (b, w, n) in runs:
                    oh = ohp.tile([128, nt_max, F], f32, tag="oh",
                                  name=f"oh_{k}_{b}_{w}")
                    nc.vector.tensor_tensor(
                        out=oh[:, :n, :],
                        in0=iot[:].unsqueeze(1).to_broadcast([128, n, F]),
                        in1=dlsb[:, tg:tg + n].unsqueeze(2)
                            .to_broadcast([128, n, F]),
                        op=mybir.AluOpType.is_equal)
                    ps = pswp.tile([128, F], f32, tag="psw")
                    for t in range(n):
                        s = slot[b] + t * F
                        msg = msg_tiles[(b, s // CHUNK)]
                        nc.tensor.matmul(ps[:], lhsT=oh[:, t, :],
                                         rhs=msg[:, (s % CHUNK) // F, :],
                                         start=(t == 0), stop=(t == n - 1))
                    if first_bank[w] == b:
                        nc.vector.tensor_copy(AGG[:, w, :], ps[:])
                    else:
                        nc.vector.tensor_add(AGG[:, w, :], AGG[:, w, :], ps[:])
                    tg += n
                    slot[b] += n * F

                # mean: scale rows by 1/deg (batched, stride-0 broadcast)
                nc.vector.tensor_tensor(
                    out=AGG[:], in0=AGG[:],
                    in1=ivd[:].unsqueeze(2).to_broadcast([128, W, F]),
                    op=mybir.AluOpType.mult)

                # Hnew = relu(W_l @ aggT + W_r @ HT + b_l), feature-major
                for w in range(W):
                    ws = slice(w * F, (w + 1) * F)
                    pt = pswp.tile([F, F], f32, tag="psw")
                    nc.tensor.transpose(pt[:], AGG[:, w, :], idn[:])
                    at = aggtp.tile([F, F], f32, tag="aggT")
                    nc.vector.tensor_copy(at[:], pt[:])
                    ph = pshp.tile([F, F], f32, tag="psh")
                    nc.tensor.matmul(ph[:], lhsT=w_l[:], rhs=at[:],
                                     start=True, stop=False)
                    nc.tensor.matmul(ph[:], lhsT=w_r[:], rhs=HT[:, ws],
                                     start=False, stop=True)
                    nc.scalar.activation(HT[:, ws], ph[:],
                                         mybir.ActivationFunctionType.Relu,
                                         bias=b_l[:, 0:1])
                    if not last or w == 0:
                        pr = psrp.tile([F, F], f32, tag="psr")
                        nc.tensor.transpose(pr[:], HT[:, ws], idn[:])
                        nc.vector.tensor_copy(AGG[:, w, :], pr[:])
                # head rows (global rows 0,1 live on core 0, window 0)
                nc.sync.dma_start(out[k, :, :], AGG[0:2, 0, :])
                if not last:
                    nc.sync.dma_start(
                        ag_in[:, :].rearrange("(w p) f -> p w f", p=128), AGG[:])
                    nc.gpsimd.collective_compute(
                        "AllGather", mybir.AluOpType.bypass, replica_groups=rg,
                        ins=[ag_in.ap().opt()], outs=[h_table.ap().opt()])

    nc.compile()
    return nc


def _heads(out_rows, W_e1, b_e1, W_e2, b_e2, W_h1, b_h1, W_h2, b_h2):
    """Host-side tiny MLP heads, mirroring the reference math in f32."""
    relu = lambda x: np.maximum(x, 0.0)
    alphas, scores = [], []
    p_not = np.float32(1.0)
    for k in range(LMAX):
        h_u = out_rows[k, 0].astype(np.float32)
        h_v = out_rows[k, 1].astype(np.float32)
        feat = np.concatenate([h_u, h_v, h_u * h_v])
        score = relu(feat @ W_e1.T + b_e1) @ W_e2.T + b_e2
        hin = np.concatenate([h_u, h_v, score])
        z = relu(hin @ W_h1.T + b_h1) @ W_h2.T + b_h2
        p_halt = np.float32(1.0) / (np.float32(1.0) + np.exp(-z[0]))
        alphas.append(p_halt * p_not)
        scores.append(score[0])
        p_not = p_not * (np.float32(1.0) - p_halt)
    alpha = np.stack(alphas).astype(np.float32)
    alpha = alpha / (alpha.sum() + np.float32(1e-8))
    scores_v = np.stack(scores).astype(np.float32)
    final_score = (alpha * scores_v).sum()
    depths = np.arange(1, LMAX + 1, dtype=np.float32)
    expected_depth = (alpha * depths).sum()
    return np.float32(final_score), np.float32(expected_depth), alpha


def _make_in_maps(inputs, x_sub, inv_deg, gidx, dls):
    W_in = np.asarray(inputs["W_in"], np.float32)
    W_l = np.asarray(inputs["W_l"], np.float32)
    W_r = np.asarray(inputs["W_r"], np.float32)
    common = dict(
        winT=np.ascontiguousarray(W_in.T),
        wlT=np.ascontiguousarray(W_l.T),
        wrT=np.ascontiguousarray(W_r.T),
        bin=np.asarray(inputs["b_in"], np.float32).reshape(F, 1),
        bl=np.asarray(inputs["b_l"], np.float32).reshape(F, 1),
        ident=np.eye(F, dtype=np.float32),
        iotar=np.tile(np.arange(F, dtype=np.float32), (F, 1)),
    )
    in_maps = []
    for c in range(N_CORES):
        rows = slice(c * PER, (c + 1) * PER)
        m = dict(common)
        m["xT"] = np.ascontiguousarray(x_sub[rows].T)
        m["invdeg"] = np.ascontiguousarray(inv_deg[rows].reshape(W, 128).T)
        m["dl"] = dls[c]
        for b in range(3):
            m[f"gidx{b}"] = gidx[c][b]
        in_maps.append(m)
    return in_maps


def _run(inputs, trace=False):
    x_full = np.asarray(inputs["x_full"], np.float32)
    subset = np.asarray(inputs["subset"], np.int64)
    ei = np.asarray(inputs["edge_index"], np.int64)
    src, dst = ei[0], ei[1]

    x_sub = np.zeros((N_PAD, F), np.float32)
    x_sub[:N_SUB] = x_full[subset]
    deg = np.maximum(np.bincount(dst, minlength=N_SUB).astype(np.float32), 1.0)
    inv_deg = np.ones(N_PAD, np.float32)
    inv_deg[:N_SUB] = 1.0 / deg

    gidx, dls, L, runs, T_total = _prep_edges(src, dst)
    nc = _build_graph(L, runs, T_total)
    in_maps = _make_in_maps(inputs, x_sub, inv_deg, gidx, dls)

    res = run_bass_kernel_spmd(nc, in_maps, list(range(N_CORES)), trace=trace)
    out_rows = np.asarray(res.results[0]["out"]).reshape(LMAX, 2, F)

    fs, ed, alpha = _heads(
        out_rows,
        np.asarray(inputs["W_e1"], np.float32), np.asarray(inputs["b_e1"], np.float32),
        np.asarray(inputs["W_e2"], np.float32), np.asarray(inputs["b_e2"], np.float32),
        np.asarray(inputs["W_h1"], np.float32), np.asarray(inputs["b_h1"], np.float32),
        np.asarray(inputs["W_h2"], np.float32), np.asarray(inputs["b_h2"], np.float32),
    )
    return (fs, ed, alpha), res


def kernel(**inputs):
    (fs, ed, alpha), _ = _run(inputs, trace=False)
    return fs, ed, alpha


# revision 12
# speedup vs baseline: 1.5954x; 1.5954x over previous
"""AdaptiveSAGE GNN message-passing kernel for 8 Trainium2 NeuronCores.

Distribution strategy (dst-sharded message passing, PE-based segment sum):
  - Subgraph nodes padded to N_PAD = 81920 = 8 * 10240; core c owns rows
    [c*10240, (c+1)*10240).
  - Edges are assigned to the core owning their destination, bucketed by
    (src bank, dst window of 128 rows), and padded per bucket to a multiple
    of 128 so all cores run one identical instruction stream.
  - Each core gathers H[src] messages with dma_gather (int16 indices =>
    the 81920-row replicated H table is split into 3 banks <= 32768 rows).
  - Segment-sum by destination runs on the TensorEngine: per 128-message
    tile a one-hot(dst) matrix (built by a batched DVE is_equal against an
    iota row) is the stationary matmul operand; PSUM accumulates the
    window's aggregate exactly (no DMA scatter -> no RMW races).
  - Per-core SAGE update is computed feature-major on the TensorEngine,
    then the new H rows are AllGathered into the replicated H table.
  - The tiny MLP heads (scores / halting probs on rows u=0, v=1) are
    evaluated on the host from the 5 x 2 x 128 head rows the kernel emits.
"""

import math

import numpy as np

import concourse.bass as bass
import concourse.bacc as bacc
import concourse.tile as tile
import concourse.mybir as mybir
from concourse.bass_utils import run_bass_kernel_spmd

F = 128          # feature dim
N_CORES = 8
N_SUB = 80000
PER = 10240      # rows per core
N_PAD = N_CORES * PER
W = PER // F     # dst windows of 128 rows per core
LMAX = 5
CHUNK = 1024     # messages per gather chunk (SWDGE ring caps num_idxs ~<2K)
PAD_DL = 999.0   # out-of-window dst marker for padding slots

# gather banks (each <= 32767 rows so indices fit int16)
BANKS = [(0, 27648), (27648, 27648), (55296, N_PAD - 55296)]


def _wrap16(idx: np.ndarray) -> np.ndarray:
    """SWDGE index layout: logical i -> [i%16, i//16], replicated across the
    8 groups of 16 partitions."""
    n = idx.shape[0]
    assert n % 16 == 0
    w = idx.reshape(n // 16, 16).T.astype(np.int16)
    return np.tile(w, (8, 1))


def _prep_edges(src: np.ndarray, dst: np.ndarray):
    """Bucket edges by (dst core, src bank, dst window); pad each bucket to a
    common multiple-of-128 so the SPMD graph is uniform across cores.

    Returns per-core gather index arrays (3 banks, wrapped int16), per-core
    one-hot dst columns dl [128, T_total] f32, per-bank padded slot counts L,
    and the static run list [(bank, window, n_tiles)].
    """
    nw = W
    core_of = dst // PER
    bank_of = np.digitize(src, [BANKS[1][0], BANKS[2][0]])
    dst_local = dst - core_of * PER
    w_of = dst_local // F
    run_of = bank_of * nw + w_of

    counts = np.zeros((N_CORES, 3 * nw), dtype=np.int64)
    per_core = []
    for c in range(N_CORES):
        m = core_of == c
        gl = (src[m] - np.array([b[0] for b in BANKS])[bank_of[m]]).astype(np.int16)
        dl = (dst_local[m] % F).astype(np.float32)
        rid = run_of[m]
        order = np.argsort(rid, kind="stable")
        gl, dl, rid = gl[order], dl[order], rid[order]
        bounds = np.searchsorted(rid, np.arange(3 * nw + 1))
        counts[c] = bounds[1:] - bounds[:-1]
        per_core.append((gl, dl, bounds))

    nt = np.ceil(counts.max(axis=0) / F).astype(np.int64)  # tiles per run
    runs = [(r // nw, r % nw, int(nt[r])) for r in range(3 * nw) if nt[r] > 0]
    # per-bank padded slot counts and per-run slot offsets
    L = [0, 0, 0]
    slot0 = {}
    tile0 = {}
    tg = 0
    for (b, w, n) in runs:
        slot0[(b, w)] = L[b]
        tile0[(b, w)] = tg
        L[b] += n * F
        tg += n
    T_total = tg

    gidx, dls = [], []
    for c in range(N_CORES):
        gl, dl, bounds = per_core[c]
        gb = [np.zeros(L[b], np.int16) for b in range(3)]
        dla = np.full(T_total * F, PAD_DL, np.float32)
        for (b, w, n) in runs:
            r = b * nw + w
            seg = slice(bounds[r], bounds[r + 1])
            cnt = bounds[r + 1] - bounds[r]
            s0 = slot0[(b, w)]
            gb[b][s0:s0 + cnt] = gl[seg]
            t0 = tile0[(b, w)]
            dla[t0 * F:t0 * F + cnt] = dl[seg]
        gidx.append([_wrap16(x) for x in gb])
        dls.append(np.ascontiguousarray(dla.reshape(T_total, F).T))
    return gidx, dls, L, runs, T_total


def _build_graph(L, runs, T_total):
    """Build the SPMD Bass graph (identical for all 8 cores)."""
    f32 = mybir.dt.float32
    bf16 = mybir.dt.bfloat16
    i16 = mybir.dt.int16
    nt_max = max(n for (_, _, n) in runs)
    # first contributing bank per window (copy vs accumulate)
    first_bank = {}
    for (b, w, n) in runs:
        first_bank.setdefault(w, b)

    nc = bacc.Bacc("TRN2", target_bir_lowering=False, debug=False,
                   num_devices=N_CORES, num_swdge_queues=4)

    # ---- kernel I/O -------------------------------------------------------
    xT = nc.dram_tensor("xT", [F, PER], f32, kind="ExternalInput")
    invdeg = nc.dram_tensor("invdeg", [F, W], f32, kind="ExternalInput")
    winT = nc.dram_tensor("winT", [F, F], f32, kind="ExternalInput")
    wlT = nc.dram_tensor("wlT", [F, F], f32, kind="ExternalInput")
    wrT = nc.dram_tensor("wrT", [F, F], f32, kind="ExternalInput")
    bin_ = nc.dram_tensor("bin", [F, 1], f32, kind="ExternalInput")
    bl = nc.dram_tensor("bl", [F, 1], f32, kind="ExternalInput")
    ident = nc.dram_tensor("ident", [F, F], f32, kind="ExternalInput")
    iotar = nc.dram_tensor("iotar", [F, F], f32, kind="ExternalInput")
    dl_d = nc.dram_tensor("dl", [F, T_total], f32, kind="ExternalInput")
    gidx_d = [nc.dram_tensor(f"gidx{b}", [128, L[b] // 16], i16,
                             kind="ExternalInput") for b in range(3)]
    out = nc.dram_tensor("out", [LMAX, 2, F], f32, kind="ExternalOutput")

    # ---- internal DRAM ----------------------------------------------------
    h_table = nc.dram_tensor("h_table", [N_PAD, F], bf16, addr_space="Shared")
    ag_in = nc.dram_tensor("ag_in", [PER, F], bf16)

    rg = [list(range(N_CORES))]

    with tile.TileContext(nc) as tc:
        with (
            tc.tile_pool(name="sb", bufs=1) as sb,
            tc.tile_pool(name="msgp", bufs=4) as msgp,
            tc.tile_pool(name="ohp", bufs=3) as ohp,
            tc.tile_pool(name="aggtp", bufs=2) as aggtp,
            tc.tile_pool(name="psw", bufs=4, space="PSUM") as pswp,
            tc.tile_pool(name="psh", bufs=2, space="PSUM") as pshp,
            tc.tile_pool(name="psr", bufs=2, space="PSUM") as psrp,
        ):
            # persistent SBUF
            HT = sb.tile([F, PER], f32, tag="HT")       # H local, feature-major
            AGG = sb.tile([128, W, F], f32, tag="AGG")  # agg rows
            HROWB = sb.tile([128, W, F], bf16, tag="HROWB")  # Hnew row-major
            headf = sb.tile([2, F], f32, tag="headf")   # rows u,v at full prec
            w_in = sb.tile([F, F], f32, tag="w_in")
            w_l = sb.tile([F, F], f32, tag="w_l")
            w_r = sb.tile([F, F], f32, tag="w_r")
            b_in = sb.tile([F, 1], f32, tag="b_in")
            b_l = sb.tile([F, 1], f32, tag="b_l")
            idn = sb.tile([F, F], f32, tag="idn")
            iot = sb.tile([F, F], f32, tag="iot")
            ivd = sb.tile([F, W], f32, tag="ivd")
            dlsb = sb.tile([F, T_total], f32, tag="dlsb")
            gsb = [sb.tile([128, L[b] // 16], i16, tag=f"g{b}", name=f"g{b}")
                   for b in range(3)]

            # ---- stage 0: loads -------------------------------------------
            nc.sync.dma_start(w_in[:], winT[:, :])
            nc.sync.dma_start(w_l[:], wlT[:, :])
            nc.sync.dma_start(w_r[:], wrT[:, :])
            nc.sync.dma_start(b_in[:], bin_[:, :])
            nc.sync.dma_start(b_l[:], bl[:, :])
            nc.sync.dma_start(idn[:], ident[:, :])
            nc.sync.dma_start(iot[:], iotar[:, :])
            nc.sync.dma_start(ivd[:], invdeg[:, :])
            nc.sync.dma_start(dlsb[:], dl_d[:, :])
            for b in range(3):
                nc.sync.dma_start(gsb[b][:], gidx_d[b][:, :])

            # xT -> AGG viewed feature-major [F, PER]
            AGGf = AGG[:].rearrange("p w f -> p (w f)")
            nc.sync.dma_start(AGGf, xT[:, :])

            # H0 = W_in @ xT + b_in (feature-major), then row-major for AG
            for w in range(W):
                ws = slice(w * F, (w + 1) * F)
                ph = pshp.tile([F, F], f32, tag="psh")
                nc.tensor.matmul(ph[:], lhsT=w_in[:], rhs=AGGf[:, ws],
                                 start=True, stop=True)
                nc.vector.tensor_scalar_add(HT[:, ws], ph[:], b_in[:, 0:1])
            for w in range(W):
                ws = slice(w * F, (w + 1) * F)
                pr = psrp.tile([F, F], f32, tag="psr")
                nc.tensor.transpose(pr[:], HT[:, ws], idn[:])
                nc.vector.tensor_copy(HROWB[:, w, :], pr[:])
            nc.sync.dma_start(ag_in[:, :].rearrange("(w p) f -> p w f", p=128),
                              HROWB[:])
            nc.gpsimd.collective_compute(
                "AllGather", mybir.AluOpType.bypass, replica_groups=rg,
                ins=[ag_in.ap().opt()], outs=[h_table.ap().opt()])

            # ---- steps ----------------------------------------------------
            for k in range(LMAX):
                last = k == LMAX - 1
                # chunked gathers, bank-major slot order, spread over the
                # 4 SWDGE queues so all gpsimd Q7 pairs generate descriptors
                msg_tiles = {}
                qn = 0
                for b in range(3):
                    base, rows = BANKS[b]
                    bank_ap = h_table[base:base + rows, :]
                    for j0 in range(0, L[b], CHUNK):
                        n = min(CHUNK, L[b] - j0)
                        msg = msgp.tile([128, CHUNK // 128, F], bf16, tag="msg",
                                        name=f"msg_{k}_{b}_{j0}")
                        cols = slice(j0 // 16, (j0 + n) // 16)
                        nc.gpsimd.dma_gather(
                            out_ap=msg[:, : n // 128, :], in_ap=bank_ap,
                            idxs_ap=gsb[b][:, cols],
                            num_idxs=n, num_idxs_reg=n, elem_size=F,
                            queue_num=qn % 4)
                        qn += 1
                        msg_tiles[(b, j0 // CHUNK)] = msg

                # segment-sum per (bank, window) run on the TensorEngine
                tg = 0
                slot = [0, 0, 0]
                for (b, w, n) in runs:
                    oh = ohp.tile([128, nt_max, F], bf16, tag="oh",
                                  name=f"oh_{k}_{b}_{w}")
                    nc.vector.tensor_tensor(
                        out=oh[:, :n, :],
                        in0=iot[:].unsqueeze(1).to_broadcast([128, n, F]),
                        in1=dlsb[:, tg:tg + n].unsqueeze(2)
                            .to_broadcast([128, n, F]),
                        op=mybir.AluOpType.is_equal)
                    ps = pswp.tile([128, F], f32, tag="psw")
                    for t in range(n):
                        s = slot[b] + t * F
                        msg = msg_tiles[(b, s // CHUNK)]
                        nc.tensor.matmul(ps[:], lhsT=oh[:, t, :],
                                         rhs=msg[:, (s % CHUNK) // F, :],
                                         start=(t == 0), stop=(t == n - 1))
                    if first_bank[w] == b:
                        nc.vector.tensor_copy(AGG[:, w, :], ps[:])
                    else:
                        nc.vector.tensor_add(AGG[:, w, :], AGG[:, w, :], ps[:])
                    tg += n
                    slot[b] += n * F

                # mean: scale rows by 1/deg (batched, stride-0 broadcast)
                nc.vector.tensor_tensor(
                    out=AGG[:], in0=AGG[:],
                    in1=ivd[:].unsqueeze(2).to_broadcast([128, W, F]),
                    op=mybir.AluOpType.mult)

                # Hnew = relu(W_l @ aggT + W_r @ HT + b_l), feature-major
                for w in range(W):
                    ws = slice(w * F, (w + 1) * F)
                    pt = pswp.tile([F, F], f32, tag="psw")
                    nc.tensor.transpose(pt[:], AGG[:, w, :], idn[:])
                    at = aggtp.tile([F, F], f32, tag="aggT")
                    nc.vector.tensor_copy(at[:], pt[:])
                    ph = pshp.tile([F, F], f32, tag="psh")
                    nc.tensor.matmul(ph[:], lhsT=w_l[:], rhs=at[:],
                                     start=True, stop=False)
                    nc.tensor.matmul(ph[:], lhsT=w_r[:], rhs=HT[:, ws],
                                     start=False, stop=True)
                    nc.scalar.activation(HT[:, ws], ph[:],
                                         mybir.ActivationFunctionType.Relu,
                                         bias=b_l[:, 0:1])
                    if not last or w == 0:
                        pr = psrp.tile([F, F], f32, tag="psr")
                        nc.tensor.transpose(pr[:], HT[:, ws], idn[:])
                        if w == 0:
                            nc.vector.tensor_copy(headf[:], pr[0:2, :])
                        if not last:
                            nc.vector.tensor_copy(HROWB[:, w, :], pr[:])
                # head rows (global rows 0,1 live on core 0, window 0)
                nc.sync.dma_start(out[k, :, :], headf[:])
                if not last:
                    nc.sync.dma_start(
                        ag_in[:, :].rearrange("(w p) f -> p w f", p=128),
                        HROWB[:])
                    nc.gpsimd.collective_compute(
                        "AllGather", mybir.AluOpType.bypass, replica_groups=rg,
                        ins=[ag_in.ap().opt()], outs=[h_table.ap().opt()])

    # Align each gather's SWDGE queue with the DMASW sem lane Tile assigned
    # (a sem lane must only ever be updated from one queue).
    import re
    for blk in nc.m.functions[0].blocks:
        for ins in blk.instructions:
            if isinstance(ins, mybir.InstDMAGatherAnt) and ins.sync_info:
                m = re.match(r"DMASW(\d+)", ins.sync_info.on_update[0].ant_name)
                if m:
                    ins.queue_num = int(m.group(1)) % 4

    nc.compile()
    return nc


def _heads(out_rows, W_e1, b_e1, W_e2, b_e2, W_h1, b_h1, W_h2, b_h2):
    """Host-side tiny MLP heads, mirroring the reference math in f32."""
    relu = lambda x: np.maximum(x, 0.0)
    alphas, scores = [], []
    p_not = np.float32(1.0)
    for k in range(LMAX):
        h_u = out_rows[k, 0].astype(np.float32)
        h_v = out_rows[k, 1].astype(np.float32)
        feat = np.concatenate([h_u, h_v, h_u * h_v])
        score = relu(feat @ W_e1.T + b_e1) @ W_e2.T + b_e2
        hin = np.concatenate([h_u, h_v, score])
        z = relu(hin @ W_h1.T + b_h1) @ W_h2.T + b_h2
        p_halt = np.float32(1.0) / (np.float32(1.0) + np.exp(-z[0]))
        alphas.append(p_halt * p_not)
        scores.append(score[0])
        p_not = p_not * (np.float32(1.0) - p_halt)
    alpha = np.stack(alphas).astype(np.float32)
    alpha = alpha / (alpha.sum() + np.float32(1e-8))
    scores_v = np.stack(scores).astype(np.float32)
    final_score = (alpha * scores_v).sum()
    depths = np.arange(1, LMAX + 1, dtype=np.float32)
    expected_depth = (alpha * depths).sum()
    return np.float32(final_score), np.float32(expected_depth), alpha


def _make_in_maps(inputs, x_sub, inv_deg, gidx, dls):
    W_in = np.asarray(inputs["W_in"], np.float32)
    W_l = np.asarray(inputs["W_l"], np.float32)
    W_r = np.asarray(inputs["W_r"], np.float32)
    common = dict(
        winT=np.ascontiguousarray(W_in.T),
        wlT=np.ascontiguousarray(W_l.T),
        wrT=np.ascontiguousarray(W_r.T),
        bin=np.asarray(inputs["b_in"], np.float32).reshape(F, 1),
        bl=np.asarray(inputs["b_l"], np.float32).reshape(F, 1),
        ident=np.eye(F, dtype=np.float32),
        iotar=np.tile(np.arange(F, dtype=np.float32), (F, 1)),
    )
    in_maps = []
    for c in range(N_CORES):
        rows = slice(c * PER, (c + 1) * PER)
        m = dict(common)
        m["xT"] = np.ascontiguousarray(x_sub[rows].T)
        m["invdeg"] = np.ascontiguousarray(inv_deg[rows].reshape(W, 128).T)
        m["dl"] = dls[c]
        for b in range(3):
            m[f"gidx{b}"] = gidx[c][b]
        in_maps.append(m)
    return in_maps


def _run(inputs, trace=False):
    x_full = np.asarray(inputs["x_full"], np.float32)
    subset = np.asarray(inputs["subset"], np.int64)
    ei = np.asarray(inputs["edge_index"], np.int64)
    src, dst = ei[0], ei[1]

    x_sub = np.zeros((N_PAD, F), np.float32)
    x_sub[:N_SUB] = x_full[subset]
    deg = np.maximum(np.bincount(dst, minlength=N_SUB).astype(np.float32), 1.0)
    inv_deg = np.ones(N_PAD, np.float32)
    inv_deg[:N_SUB] = 1.0 / deg

    gidx, dls, L, runs, T_total = _prep_edges(src, dst)
    nc = _build_graph(L, runs, T_total)
    in_maps = _make_in_maps(inputs, x_sub, inv_deg, gidx, dls)

    res = run_bass_kernel_spmd(nc, in_maps, list(range(N_CORES)), trace=trace)
    out_rows = np.asarray(res.results[0]["out"]).reshape(LMAX, 2, F)

    fs, ed, alpha = _heads(
        out_rows,
        np.asarray(inputs["W_e1"], np.float32), np.asarray(inputs["b_e1"], np.float32),
        np.asarray(inputs["W_e2"], np.float32), np.asarray(inputs["b_e2"], np.float32),
        np.asarray(inputs["W_h1"], np.float32), np.asarray(inputs["b_h1"], np.float32),
        np.asarray(inputs["W_h2"], np.float32), np.asarray(inputs["b_h2"], np.float32),
    )
    return (fs, ed, alpha), res


def kernel(**inputs):
    (fs, ed, alpha), _ = _run(inputs, trace=False)
    return fs, ed, alpha


# revision 17
# speedup vs baseline: 2.6304x; 1.6487x over previous
"""AdaptiveSAGE GNN message-passing kernel for 8 Trainium2 NeuronCores.

Distribution strategy (dst-sharded message passing, PE-based segment sum):
  - Subgraph nodes padded to N_PAD = 81920 = 8 * 10240; core c owns rows
    [c*10240, (c+1)*10240).
  - Edges are assigned to the core owning their destination, bucketed by
    (src bank, dst window of 128 rows), and padded per bucket to a multiple
    of 128 so all cores run one identical instruction stream.
  - Each core gathers H[src] messages with dma_gather (int16 indices =>
    the 81920-row replicated H table is split into 3 banks <= 32768 rows).
  - Segment-sum by destination runs on the TensorEngine: per 128-message
    tile a one-hot(dst) matrix (built by a batched DVE is_equal against an
    iota row) is the stationary matmul operand; PSUM accumulates the
    window's aggregate exactly (no DMA scatter -> no RMW races).
  - Per-core SAGE update is computed feature-major on the TensorEngine,
    then the new H rows are AllGathered into the replicated H table.
  - The tiny MLP heads (scores / halting probs on rows u=0, v=1) are
    evaluated on the host from the 5 x 2 x 128 head rows the kernel emits.
"""

import math

import numpy as np

import concourse.bass as bass
import concourse.bacc as bacc
import concourse.tile as tile
import concourse.mybir as mybir
from concourse.bass_utils import run_bass_kernel_spmd

F = 128          # feature dim
N_CORES = 8
N_SUB = 80000
PER = 10240      # rows per core
N_PAD = N_CORES * PER
W = PER // F     # dst windows of 128 rows per core
LMAX = 5
CHUNK = 1024     # messages per gather chunk (SWDGE ring caps num_idxs ~<2K)
PAD_DL = 999.0   # out-of-window dst marker for padding slots

# gather banks (each <= 32767 rows so indices fit int16)
BANKS = [(0, 27648), (27648, 27648), (55296, N_PAD - 55296)]


def _wrap16(idx: np.ndarray) -> np.ndarray:
    """SWDGE index layout: logical i -> [i%16, i//16], replicated across the
    8 groups of 16 partitions."""
    n = idx.shape[0]
    assert n % 16 == 0
    w = idx.reshape(n // 16, 16).T.astype(np.int16)
    return np.tile(w, (8, 1))


def _prep_edges(src: np.ndarray, dst: np.ndarray):
    """Bucket edges by (dst core, src bank, dst window); pad each bucket to a
    common multiple-of-128 so the SPMD graph is uniform across cores.

    Returns per-core gather index arrays (3 banks, wrapped int16), per-core
    one-hot dst columns dl [128, T_total] f32, per-bank padded slot counts L,
    and the static run list [(bank, window, n_tiles)].
    """
    nw = W
    core_of = dst // PER
    bank_of = np.digitize(src, [BANKS[1][0], BANKS[2][0]])
    dst_local = dst - core_of * PER
    w_of = dst_local // F
    run_of = bank_of * nw + w_of

    counts = np.zeros((N_CORES, 3 * nw), dtype=np.int64)
    per_core = []
    for c in range(N_CORES):
        m = core_of == c
        gl = (src[m] - np.array([b[0] for b in BANKS])[bank_of[m]]).astype(np.int16)
        dl = (dst_local[m] % F).astype(np.float32)
        rid = run_of[m]
        order = np.argsort(rid, kind="stable")
        gl, dl, rid = gl[order], dl[order], rid[order]
        bounds = np.searchsorted(rid, np.arange(3 * nw + 1))
        counts[c] = bounds[1:] - bounds[:-1]
        per_core.append((gl, dl, bounds))

    nt = np.ceil(counts.max(axis=0) / F).astype(np.int64)  # tiles per run
    # window-major run order (w, b): a window's three bank runs are
    # consecutive, so one PSUM tile accumulates them all
    runs = [(w, b, int(nt[b * nw + w]))
            for w in range(nw) for b in range(3) if nt[b * nw + w] > 0]
    # per-bank padded slot counts (bank-major gather layout, window-sorted
    # within each bank) and per-run tile offsets in (w, b) order
    L = [0, 0, 0]
    slot0 = {}
    for b in range(3):
        for w in range(nw):
            n = int(nt[b * nw + w])
            if n:
                slot0[(b, w)] = L[b]
                L[b] += n * F
    tile0 = {}
    tg = 0
    for (w, b, n) in runs:
        tile0[(b, w)] = tg
        tg += n
    T_total = tg

    gidx, dls = [], []
    for c in range(N_CORES):
        gl, dl, bounds = per_core[c]
        gb = [np.zeros(L[b], np.int16) for b in range(3)]
        dla = np.full(T_total * F, PAD_DL, np.float32)
        for (w, b, n) in runs:
            r = b * nw + w
            seg = slice(bounds[r], bounds[r + 1])
            cnt = bounds[r + 1] - bounds[r]
            s0 = slot0[(b, w)]
            gb[b][s0:s0 + cnt] = gl[seg]
            t0 = tile0[(b, w)]
            dla[t0 * F:t0 * F + cnt] = dl[seg]
        gidx.append([_wrap16(x) for x in gb])
        dls.append(np.ascontiguousarray(dla.reshape(T_total, F).T))
    meta = dict(L=L, runs=runs, T_total=T_total, slot0=slot0, tile0=tile0)
    return gidx, dls, meta


def _build_graph(meta):
    L = meta["L"]
    runs = meta["runs"]
    T_total = meta["T_total"]
    slot0 = meta["slot0"]
    tile0 = meta["tile0"]
    """Build the SPMD Bass graph (identical for all 8 cores)."""
    f32 = mybir.dt.float32
    bf16 = mybir.dt.bfloat16
    i16 = mybir.dt.int16
    nc = bacc.Bacc("TRN2", target_bir_lowering=False, debug=False,
                   num_devices=N_CORES, num_swdge_queues=4)

    # ---- kernel I/O -------------------------------------------------------
    xT = nc.dram_tensor("xT", [F, PER], f32, kind="ExternalInput")
    invdeg = nc.dram_tensor("invdeg", [F, W], f32, kind="ExternalInput")
    winT = nc.dram_tensor("winT", [F, F], f32, kind="ExternalInput")
    wlT = nc.dram_tensor("wlT", [F, F], f32, kind="ExternalInput")
    wrT = nc.dram_tensor("wrT", [F, F], f32, kind="ExternalInput")
    bin_ = nc.dram_tensor("bin", [F, 1], f32, kind="ExternalInput")
    bl = nc.dram_tensor("bl", [F, 1], f32, kind="ExternalInput")
    ident = nc.dram_tensor("ident", [F, F], f32, kind="ExternalInput")
    iotar = nc.dram_tensor("iotar", [F, F], f32, kind="ExternalInput")
    dl_d = nc.dram_tensor("dl", [F, T_total], f32, kind="ExternalInput")
    gidx_d = [nc.dram_tensor(f"gidx{b}", [128, L[b] // 16], i16,
                             kind="ExternalInput") for b in range(3)]
    out = nc.dram_tensor("out", [LMAX, 2, F], f32, kind="ExternalOutput")

    # ---- internal DRAM ----------------------------------------------------
    h_table = nc.dram_tensor("h_table", [N_PAD, F], bf16, addr_space="Shared")
    ag_in = nc.dram_tensor("ag_in", [PER, F], bf16)

    rg = [list(range(N_CORES))]

    with tile.TileContext(nc) as tc:
        with (
            tc.tile_pool(name="sb", bufs=1) as sb,
            tc.tile_pool(name="msgp", bufs=8) as msgp,
            tc.tile_pool(name="ohp", bufs=4) as ohp,
            tc.tile_pool(name="aggtp", bufs=2) as aggtp,
            tc.tile_pool(name="psw", bufs=4, space="PSUM") as pswp,
            tc.tile_pool(name="psh", bufs=2, space="PSUM") as pshp,
            tc.tile_pool(name="psr", bufs=2, space="PSUM") as psrp,
        ):
            # persistent SBUF
            HT = sb.tile([F, PER], f32, tag="HT")       # H local, feature-major
            AGG = sb.tile([128, W, F], f32, tag="AGG")  # agg rows
            HROWB = sb.tile([128, W, F], bf16, tag="HROWB")  # Hnew row-major
            headf = sb.tile([2, F], f32, tag="headf")   # rows u,v at full prec
            w_in = sb.tile([F, F], f32, tag="w_in")
            w_l = sb.tile([F, F], f32, tag="w_l")
            w_r = sb.tile([F, F], f32, tag="w_r")
            b_in = sb.tile([F, 1], f32, tag="b_in")
            b_l = sb.tile([F, 1], f32, tag="b_l")
            idn = sb.tile([F, F], f32, tag="idn")
            iot = sb.tile([F, F], f32, tag="iot")
            ivd = sb.tile([F, W], f32, tag="ivd")
            dlsb = sb.tile([F, T_total], f32, tag="dlsb")
            gsb = [sb.tile([128, L[b] // 16], i16, tag=f"g{b}", name=f"g{b}")
                   for b in range(3)]

            # ---- stage 0: loads -------------------------------------------
            nc.sync.dma_start(w_in[:], winT[:, :])
            nc.sync.dma_start(w_l[:], wlT[:, :])
            nc.sync.dma_start(w_r[:], wrT[:, :])
            nc.sync.dma_start(b_in[:], bin_[:, :])
            nc.sync.dma_start(b_l[:], bl[:, :])
            nc.sync.dma_start(idn[:], ident[:, :])
            nc.sync.dma_start(iot[:], iotar[:, :])
            nc.sync.dma_start(ivd[:], invdeg[:, :])
            nc.sync.dma_start(dlsb[:], dl_d[:, :])
            for b in range(3):
                nc.sync.dma_start(gsb[b][:], gidx_d[b][:, :])

            # xT -> AGG viewed feature-major [F, PER]
            AGGf = AGG[:].rearrange("p w f -> p (w f)")
            nc.sync.dma_start(AGGf, xT[:, :])

            # H0 = W_in @ xT + b_in (feature-major), then row-major for AG
            for w in range(W):
                ws = slice(w * F, (w + 1) * F)
                ph = pshp.tile([F, F], f32, tag="psh")
                nc.tensor.matmul(ph[:], lhsT=w_in[:], rhs=AGGf[:, ws],
                                 start=True, stop=True)
                nc.vector.tensor_scalar_add(HT[:, ws], ph[:], b_in[:, 0:1])
            for w in range(W):
                ws = slice(w * F, (w + 1) * F)
                pr = psrp.tile([F, F], f32, tag="psr")
                nc.tensor.transpose(pr[:], HT[:, ws], idn[:])
                nc.vector.tensor_copy(HROWB[:, w, :], pr[:])
            nc.sync.dma_start(ag_in[:, :].rearrange("(w p) f -> p w f", p=128),
                              HROWB[:])
            nc.gpsimd.collective_compute(
                "AllGather", mybir.AluOpType.bypass, replica_groups=rg,
                ins=[ag_in.ap().opt()], outs=[h_table.ap().opt()])

            # ---- steps ----------------------------------------------------
            win_runs = {}
            for (w, b, n) in runs:
                win_runs.setdefault(w, []).append((b, n))
            ntw_max = max(sum(n for (_, n) in rr) for rr in win_runs.values())

            for k in range(LMAX):
                last = k == LMAX - 1
                msg_tiles = {}

                def ensure_chunk(b, j0, k=k, msg_tiles=msg_tiles):
                    if (b, j0) in msg_tiles:
                        return msg_tiles[(b, j0)]
                    n = min(CHUNK, L[b] - j0)
                    msg = msgp.tile([128, CHUNK // 128, F], bf16, tag="msg",
                                    name=f"msg_{k}_{b}_{j0}")
                    cols = slice(j0 // 16, (j0 + n) // 16)
                    nc.gpsimd.dma_gather(
                        out_ap=msg[:, : n // 128, :],
                        in_ap=h_table[BANKS[b][0]:BANKS[b][0] + BANKS[b][1], :],
                        idxs_ap=gsb[b][:, cols],
                        num_idxs=n, num_idxs_reg=n, elem_size=F)
                    msg_tiles[(b, j0)] = msg
                    return msg

                # window-major: segment-sum + SAGE update per window, so the
                # update pipeline runs underneath the gather stream
                for w in range(W):
                    ws = slice(w * F, (w + 1) * F)
                    rr = win_runs.get(w, [])
                    if rr:
                        ntw = sum(n for (_, n) in rr)
                        tg0 = tile0[(rr[0][0], w)]
                        oh = ohp.tile([128, ntw_max, F], bf16, tag="oh",
                                      name=f"oh_{k}_{w}")
                        nc.vector.tensor_tensor(
                            out=oh[:, :ntw, :],
                            in0=iot[:].unsqueeze(1).to_broadcast([128, ntw, F]),
                            in1=dlsb[:, tg0:tg0 + ntw].unsqueeze(2)
                                .to_broadcast([128, ntw, F]),
                            op=mybir.AluOpType.is_equal)
                        ps = pswp.tile([128, F], f32, tag="psw")
                        ti = 0
                        for (b, n) in rr:
                            s0 = slot0[(b, w)]
                            for t in range(n):
                                s = s0 + t * F
                                msg = ensure_chunk(b, (s // CHUNK) * CHUNK)
                                nc.tensor.matmul(
                                    ps[:], lhsT=oh[:, ti, :],
                                    rhs=msg[:, (s % CHUNK) // F, :],
                                    start=(ti == 0), stop=(ti == ntw - 1))
                                ti += 1
                        # evacuate with the 1/deg scaling folded in
                        nc.vector.tensor_scalar_mul(AGG[:, w, :], ps[:],
                                                    ivd[:, w:w + 1])
                    else:
                        nc.vector.memset(AGG[:, w, :], 0.0)

                    # Hnew_w = relu(W_l @ aggT + W_r @ HT + b_l)
                    pt = pswp.tile([F, F], f32, tag="psw")
                    nc.tensor.transpose(pt[:], AGG[:, w, :], idn[:])
                    at = aggtp.tile([F, F], f32, tag="aggT")
                    nc.vector.tensor_copy(at[:], pt[:])
                    ph = pshp.tile([F, F], f32, tag="psh")
                    nc.tensor.matmul(ph[:], lhsT=w_l[:], rhs=at[:],
                                     start=True, stop=False)
                    nc.tensor.matmul(ph[:], lhsT=w_r[:], rhs=HT[:, ws],
                                     start=False, stop=True)
                    nc.scalar.activation(HT[:, ws], ph[:],
                                         mybir.ActivationFunctionType.Relu,
                                         bias=b_l[:, 0:1])
                    if not last or w == 0:
                        pr = psrp.tile([F, F], f32, tag="psr")
                        nc.tensor.transpose(pr[:], HT[:, ws], idn[:])
                        if w == 0:
                            nc.vector.tensor_copy(headf[:], pr[0:2, :])
                        if not last:
                            nc.vector.tensor_copy(HROWB[:, w, :], pr[:])
                # head rows (global rows 0,1 live on core 0, window 0)
                nc.sync.dma_start(out[k, :, :], headf[:])
                if not last:
                    nc.sync.dma_start(
                        ag_in[:, :].rearrange("(w p) f -> p w f", p=128),
                        HROWB[:])
                    nc.gpsimd.collective_compute(
                        "AllGather", mybir.AluOpType.bypass, replica_groups=rg,
                        ins=[ag_in.ap().opt()], outs=[h_table.ap().opt()])

    # Align each gather's SWDGE queue with the DMASW sem lane Tile assigned
    # (a sem lane must only ever be updated from one queue).
    import re
    for blk in nc.m.functions[0].blocks:
        for ins in blk.instructions:
            if isinstance(ins, mybir.InstDMAGatherAnt) and ins.sync_info:
                m = re.match(r"DMASW(\d+)", ins.sync_info.on_update[0].ant_name)
                if m:
                    ins.queue_num = int(m.group(1)) % 4

    nc.compile()
    return nc


def _heads(out_rows, W_e1, b_e1, W_e2, b_e2, W_h1, b_h1, W_h2, b_h2):
    """Host-side tiny MLP heads, mirroring the reference math in f32."""
    relu = lambda x: np.maximum(x, 0.0)
    alphas, scores = [], []
    p_not = np.float32(1.0)
    for k in range(LMAX):
        h_u = out_rows[k, 0].astype(np.float32)
        h_v = out_rows[k, 1].astype(np.float32)
        feat = np.concatenate([h_u, h_v, h_u * h_v])
        score = relu(feat @ W_e1.T + b_e1) @ W_e2.T + b_e2
        hin = np.concatenate([h_u, h_v, score])
        z = relu(hin @ W_h1.T + b_h1) @ W_h2.T + b_h2
        p_halt = np.float32(1.0) / (np.float32(1.0) + np.exp(-z[0]))
        alphas.append(p_halt * p_not)
        scores.append(score[0])
        p_not = p_not * (np.float32(1.0) - p_halt)
    alpha = np.stack(alphas).astype(np.float32)
    alpha = alpha / (alpha.sum() + np.float32(1e-8))
    scores_v = np.stack(scores).astype(np.float32)
    final_score = (alpha * scores_v).sum()
    depths = np.arange(1, LMAX + 1, dtype=np.float32)
    expected_depth = (alpha * depths).sum()
    return np.float32(final_score), np.float32(expected_depth), alpha


def _make_in_maps(inputs, x_sub, inv_deg, gidx, dls):
    W_in = np.asarray(inputs["W_in"], np.float32)
    W_l = np.asarray(inputs["W_l"], np.float32)
    W_r = np.asarray(inputs["W_r"], np.float32)
    common = dict(
        winT=np.ascontiguousarray(W_in.T),
        wlT=np.ascontiguousarray(W_l.T),
        wrT=np.ascontiguousarray(W_r.T),
        bin=np.asarray(inputs["b_in"], np.float32).reshape(F, 1),
        bl=np.asarray(inputs["b_l"], np.float32).reshape(F, 1),
        ident=np.eye(F, dtype=np.float32),
        iotar=np.tile(np.arange(F, dtype=np.float32), (F, 1)),
    )
    in_maps = []
    for c in range(N_CORES):
        rows = slice(c * PER, (c + 1) * PER)
        m = dict(common)
        m["xT"] = np.ascontiguousarray(x_sub[rows].T)
        m["invdeg"] = np.ascontiguousarray(inv_deg[rows].reshape(W, 128).T)
        m["dl"] = dls[c]
        for b in range(3):
            m[f"gidx{b}"] = gidx[c][b]
        in_maps.append(m)
    return in_maps


def _run(inputs, trace=False):
    x_full = np.asarray(inputs["x_full"], np.float32)
    subset = np.asarray(inputs["subset"], np.int64)
    ei = np.asarray(inputs["edge_index"], np.int64)
    src, dst = ei[0], ei[1]

    x_sub = np.zeros((N_PAD, F), np.float32)
    x_sub[:N_SUB] = x_full[subset]
    deg = np.maximum(np.bincount(dst, minlength=N_SUB).astype(np.float32), 1.0)
    inv_deg = np.ones(N_PAD, np.float32)
    inv_deg[:N_SUB] = 1.0 / deg

    gidx, dls, meta = _prep_edges(src, dst)
    nc = _build_graph(meta)
    in_maps = _make_in_maps(inputs, x_sub, inv_deg, gidx, dls)

    res = run_bass_kernel_spmd(nc, in_maps, list(range(N_CORES)), trace=trace)
    out_rows = np.asarray(res.results[0]["out"]).reshape(LMAX, 2, F)

    fs, ed, alpha = _heads(
        out_rows,
        np.asarray(inputs["W_e1"], np.float32), np.asarray(inputs["b_e1"], np.float32),
        np.asarray(inputs["W_e2"], np.float32), np.asarray(inputs["b_e2"], np.float32),
        np.asarray(inputs["W_h1"], np.float32), np.asarray(inputs["b_h1"], np.float32),
        np.asarray(inputs["W_h2"], np.float32), np.asarray(inputs["b_h2"], np.float32),
    )
    return (fs, ed, alpha), res


def kernel(**inputs):
    (fs, ed, alpha), _ = _run(inputs, trace=False)
    return fs, ed, alpha
